# revision 12
# baseline (speedup 1.0000x reference)
"""Local causal (sliding-window) attention block on 8 TRN2 NeuronCores.

Reference computation (per batch b):
    h = LayerNorm(x) * gamma + beta
    Q = h@Wq, K = h@Wk, V = h@Wv          (heads: 16 x 64)
    S = QK^T/sqrt(dk) masked to causal band of width 256
    out = x + softmax(S)@V @ Wo + bo

Sharding: 8 cores = 2 batches x 4 head-groups (4 heads each).
Each core receives x^T (feature-major, bf16) for its batch, computes LN
stats via ones-matmuls (mu, E[x^2] broadcast over partitions), centers
and scales x^T on DVE, projects Q^T/K^T (feature-major) and V
(token-major), then does banded attention in the transposed-score
domain: S^T = K_blk^T Q (so softmaxed probabilities are directly in
the layout the P^T V matmul needs -- no PE transposes).  The softmax
denominator comes from ones-vector matmuls over P^T, is reciprocal'd
and broadcast back over partitions with a K=1 matmul, and the
normalization is applied per-head on the attention output.  The
partial out-projection  attn_g @ Wo[g]  is written token-major in
bf16.  Host reduces: out[b] = x[b] + sum_g partial[b,g] + bo.

Everything on the PE runs in bf16 (FWL fast weight loads); gamma (and
1/sqrt(dk) for Q) are folded into the projection weights on the host;
beta enters via folded bias vectors beta@W.
"""

import os

import numpy as np

import concourse.bass as bass
import concourse.tile as tile
from concourse import bacc, mybir
from concourse.bass_utils import run_bass_kernel_spmd

F32 = mybir.dt.float32
BF16 = mybir.dt.bfloat16

T = 2048          # tokens per batch
D = 1024          # model dim
HG = 4            # heads per core
DK = 64           # head dim
DG = HG * DK      # head-group feature width (256)
WIN = 256         # attention window
P = 128           # partitions
NT = T // P       # 16 token tiles
KC = D // P       # 8 feature chunks
NQ = 512          # projection tile width
NTQ = T // NQ     # 4
LN_EPS = 1e-5

# filled by test.py via run(trace=True)
LAST_PROFILE = {}


def _body(tc):
    nc = tc.nc
    with nc.allow_low_precision(reason="bf16 pipeline; rel-err budget 2e-2"):
        _body_inner(tc)


def _body_inner(tc):
    nc = tc.nc

    xt = nc.dram_tensor("xt", [D, T], BF16, kind="ExternalInput").ap()
    wq = nc.dram_tensor("wq", [D, DG], BF16, kind="ExternalInput").ap()
    wk = nc.dram_tensor("wk", [D, DG], BF16, kind="ExternalInput").ap()
    wv = nc.dram_tensor("wv", [D, DG], BF16, kind="ExternalInput").ap()
    wo = nc.dram_tensor("wo", [DG, D], BF16, kind="ExternalInput").ap()
    bq = nc.dram_tensor("bq", [P, 2], F32, kind="ExternalInput").ap()
    bk = nc.dram_tensor("bk", [P, 2], F32, kind="ExternalInput").ap()
    bvb = nc.dram_tensor("bvb", [P, DG], F32, kind="ExternalInput").ap()
    m01 = nc.dram_tensor("m01", [P, 2 * 3 * P], BF16, kind="ExternalInput").ap()
    partial = nc.dram_tensor("partial", [T, D], BF16, kind="ExternalOutput").ap()

    with (
        tc.tile_pool(name="consts", bufs=1) as consts,
        tc.tile_pool(name="big", bufs=1) as big,
    ):
        # ---- resident SBUF tensors ----
        xc = [consts.tile([P, T], BF16, tag=f"x{c}", name=f"x{c}")
              for c in range(KC)]
        for c in range(KC):
            nc.sync.dma_start(out=xc[c], in_=xt[c * P:(c + 1) * P, :])

        wq_sb = consts.tile([P, KC, DG], BF16, tag="wq")
        wk_sb = consts.tile([P, KC, DG], BF16, tag="wk")
        wv_sb = consts.tile([P, KC, DG], BF16, tag="wv")
        wo_sb = consts.tile([P, 2, D], BF16, tag="wo")
        bq_sb = consts.tile([P, 2], F32, tag="bq")
        bk_sb = consts.tile([P, 2], F32, tag="bk")
        bvb_sb = consts.tile([P, DG], F32, tag="bvb")
        m01_sb = consts.tile([P, 2, 3 * P], BF16, tag="m01")
        nc.sync.dma_start(out=wq_sb, in_=wq.rearrange("(c p) n -> p c n", p=P))
        nc.sync.dma_start(out=wk_sb, in_=wk.rearrange("(c p) n -> p c n", p=P))
        nc.sync.dma_start(out=wv_sb, in_=wv.rearrange("(c p) n -> p c n", p=P))
        nc.sync.dma_start(out=wo_sb, in_=wo.rearrange("(c p) n -> p c n", p=P))
        nc.sync.dma_start(out=bq_sb, in_=bq)
        nc.sync.dma_start(out=bk_sb, in_=bk)
        nc.sync.dma_start(out=bvb_sb, in_=bvb)
        nc.sync.dma_start(out=m01_sb, in_=m01.rearrange("p (h n) -> p h n", h=2))

        ones_mu = consts.tile([P, P], BF16, tag="ones_mu")
        onescol = consts.tile([P, 1], BF16, tag="onescol")
        ones1 = consts.tile([1, P], BF16, tag="ones1")
        eps_sb = consts.tile([P, 1], F32, tag="eps")
        nc.vector.memset(ones_mu, 1.0 / D)
        nc.vector.memset(onescol, 1.0)
        nc.vector.memset(ones1, 1.0)
        nc.vector.memset(eps_sb, LN_EPS)

        mu_sb = big.tile([P, T], BF16, tag="mu")
        rstd_sb = big.tile([P, T], BF16, tag="rstd")
        xs = [big.tile([P, T], BF16, tag=f"xs{c}", name=f"xs{c}")
              for c in range(KC)]
        qt_sb = big.tile([P, 2, T], BF16, tag="qt")
        kt_sb = big.tile([P, 2, T], BF16, tag="kt")
        v_sb = big.tile([P, NT, DG], BF16, tag="v")
        ot_sb = big.tile([P, 2, T], BF16, tag="ot")

        # ==== Phases A+B: LN stats (via matmul), xs, Q/K/V projections ====
        # Processed in two token halves so stats matmuls of half h+1
        # overlap the DVE evac/xs work of half h (keeps PE dense/warm).
        from concourse.dve_ops import (
            RECIP_APPROX_FAST_CONSTS,
            RECIPROCAL_APPROX_FAST,
        )
        rc = RECIP_APPROX_FAST_CONSTS
        TH = T // 2
        with (
            tc.tile_pool(name="statp", bufs=1, space="PSUM") as statp,
            tc.tile_pool(name="sqp", bufs=3) as sqp,
            tc.tile_pool(name="stmp", bufs=2) as stmp,
            tc.tile_pool(name="qkp", bufs=2, space="PSUM") as qkp,
            tc.tile_pool(name="vp", bufs=2, space="PSUM") as vp,
        ):
            for h in range(2):
                hsl = slice(h * TH, (h + 1) * TH)
                mu_t = statp.tile([P, 2, NQ], F32, tag="mu", name=f"mu{h}")
                msq_t = statp.tile([P, 2, NQ], F32, tag="msq",
                                   name=f"msq{h}")
                for c in range(KC):
                    sqt = sqp.tile([P, TH], BF16, tag="sq")
                    nc.vector.tensor_mul(sqt, xc[c][:, hsl], xc[c][:, hsl])
                    for k in range(2):
                        nt = 2 * h + k
                        sl = slice(nt * NQ, (nt + 1) * NQ)
                        nc.tensor.matmul(
                            mu_t[:, k, :], ones_mu, xc[c][:, sl],
                            start=(c == 0), stop=(c == KC - 1),
                            skip_group_check=True,
                        )
                        nc.tensor.matmul(
                            msq_t[:, k, :], ones_mu,
                            sqt[:, k * NQ:(k + 1) * NQ],
                            start=(c == 0), stop=(c == KC - 1),
                            skip_group_check=True,
                        )
                for k in range(2):
                    nt = 2 * h + k
                    sl = slice(nt * NQ, (nt + 1) * NQ)
                    nc.vector.tensor_copy(mu_sb[:, sl], mu_t[:, k, :])
                    m2 = stmp.tile([P, NQ], F32, tag="m2")
                    nc.vector.tensor_mul(m2, mu_sb[:, sl], mu_sb[:, sl])
                    var = stmp.tile([P, NQ], F32, tag="var")
                    nc.vector.tensor_sub(var, msq_t[:, k, :], m2)
                    sd = stmp.tile([P, NQ], F32, tag="sd")
                    nc.scalar.activation(
                        out=sd, in_=var,
                        func=mybir.ActivationFunctionType.Sqrt,
                        bias=eps_sb, scale=1.0,
                    )
                    nc.vector._custom_dve(
                        RECIPROCAL_APPROX_FAST,
                        out=rstd_sb[:, sl], in0=sd,
                        s0=rc["s0"], s1=rc["s1"], imm2=rc["imm2"],
                    )
                # xs = (x - mu) * rstd   (normalized input, bf16)
                for c in range(KC):
                    nc.vector.tensor_sub(
                        xs[c][:, hsl], xc[c][:, hsl], mu_sb[:, hsl])
                    nc.vector.tensor_mul(
                        xs[c][:, hsl], xs[c][:, hsl], rstd_sb[:, hsl])
                # Q/K for this half
                for k in range(2):
                    nt = 2 * h + k
                    sl = slice(nt * NQ, (nt + 1) * NQ)
                    for oc in range(2):
                        for w_sb, dst, b_sb in ((wq_sb, qt_sb, bq_sb),
                                                (wk_sb, kt_sb, bk_sb)):
                            ps = qkp.tile([P, NQ], F32, tag="ps")
                            for c in range(KC):
                                nc.tensor.matmul(
                                    ps,
                                    w_sb[:, c, oc * P:(oc + 1) * P],
                                    xs[c][:, sl],
                                    start=(c == 0), stop=(c == KC - 1),
                                )
                            nc.scalar.activation(
                                out=dst[:, oc, sl], in_=ps,
                                func=mybir.ActivationFunctionType.Identity,
                                bias=b_sb[:, oc:oc + 1], scale=1.0,
                            )
            for tb in range(NT):
                tsl = slice(tb * P, (tb + 1) * P)
                ps = vp.tile([P, DG], F32, tag="psv")
                for c in range(KC):
                    nc.tensor.matmul(
                        ps, xs[c][:, tsl], wv_sb[:, c, :],
                        start=(c == 0), stop=(c == KC - 1),
                    )
                nc.vector.tensor_add(v_sb[:, tb, :], ps, bvb_sb)

        # ====== Phase C+D: banded attention (transposed scores) + out ====
        with (
            tc.tile_pool(name="sp", bufs=2, space="PSUM") as sp,
            tc.tile_pool(name="avp", bufs=2, space="PSUM") as avp,
            tc.tile_pool(name="dnp", bufs=2, space="PSUM") as dnp,
            tc.tile_pool(name="dps", bufs=2, space="PSUM") as dps,
            tc.tile_pool(name="pbp", bufs=8) as pbp,
            tc.tile_pool(name="dvp", bufs=3) as dvp,
            tc.tile_pool(name="obp", bufs=3) as obp,
        ):
            pbs = {}
            for qb in range(NT):
                qsl = slice(qb * P, (qb + 1) * P)
                for oc in range(2):
                    # scores S^T for key-block jb=qb against i in
                    # [qb*P, qb*P+ni), exp'd and 0/1-masked
                    jb = qb
                    i0 = jb * P
                    ni = min(3 * P, T - i0)
                    pb = pbp.tile([P, 2, 3 * P], BF16, tag="pb")
                    pbs[(oc, jb)] = pb
                    for hh in range(2):
                        p0 = hh * DK
                        st = sp.tile([P, NQ], F32, tag="st")
                        nc.tensor.matmul(
                            st[:, :ni],
                            kt_sb[p0:p0 + DK, oc, jb * P:(jb + 1) * P],
                            qt_sb[p0:p0 + DK, oc, i0:i0 + ni],
                            start=True, stop=True,
                        )
                        nc.scalar.activation(
                            out=pb[:, hh, :ni], in_=st[:, :ni],
                            func=mybir.ActivationFunctionType.Exp,
                        )
                    nc.gpsimd.tensor_mul(
                        pb[:, :, :ni], pb[:, :, :ni], m01_sb[:, :, :ni])

                    # softmax denominator + P^T V for query-block qb
                    jlo = max(0, qb - 2)
                    njb = qb - jlo + 1
                    av = avp.tile([P, P], F32, tag="av")
                    den = dnp.tile([1, 2, P], F32, tag="den")
                    for k, j in enumerate(range(jlo, qb + 1)):
                        pbj = pbs[(oc, j)]
                        c0 = (qb - j) * P
                        nc.tensor.matmul(
                            den, onescol, pbj[:, :, c0:c0 + P],
                            start=(k == 0), stop=(k == njb - 1),
                        )
                    dinv = dvp.tile([1, 2, P], BF16, tag="dinv")
                    nc.vector._custom_dve(
                        RECIPROCAL_APPROX_FAST,
                        out=dinv, in0=den,
                        s0=rc["s0"], s1=rc["s1"], imm2=rc["imm2"],
                    )
                    dinvb = dvp.tile([P, 2, P], BF16, tag="dinvb")
                    nc.gpsimd.partition_broadcast(dinvb, dinv)
                    for hh in range(2):
                        p0 = hh * DK
                        for k, j in enumerate(range(jlo, qb + 1)):
                            pbj = pbs[(oc, j)]
                            c0 = (qb - j) * P
                            nc.tensor.matmul(
                                av[p0:p0 + DK, :],
                                v_sb[:, j, oc * P + p0:oc * P + p0 + DK],
                                pbj[:, hh, c0:c0 + P],
                                start=(k == 0), stop=(k == njb - 1),
                                tile_position=(0, p0),
                            )
                    for hh in range(2):
                        p0 = hh * DK
                        nc.vector.tensor_mul(
                            ot_sb[p0:p0 + DK, oc, qsl],
                            av[p0:p0 + DK, :],
                            dinvb[p0:p0 + DK, hh, :],
                        )

                # out-projection for this token block
                for on in range(2):
                    ps = dps.tile([P, NQ], F32, tag="ps")
                    for kd in range(2):
                        nc.tensor.matmul(
                            ps,
                            ot_sb[:, kd, qsl],
                            wo_sb[:, kd, on * NQ:(on + 1) * NQ],
                            start=(kd == 0), stop=(kd == 1),
                        )
                    ob = obp.tile([P, NQ], BF16, tag="ob")
                    if on == 0:
                        nc.vector.tensor_copy(ob, ps)
                    else:
                        nc.scalar.activation(
                            out=ob, in_=ps,
                            func=mybir.ActivationFunctionType.Copy,
                        )
                    nc.sync.dma_start(
                        out=partial[qsl, on * NQ:(on + 1) * NQ], in_=ob)


def build_nc():
    nc = bacc.Bacc("TRN2", target_bir_lowering=False, debug=False,
                   num_devices=8)
    with tile.TileContext(nc) as tc:
        _body(tc)
    nc.compile()
    return nc


def _prep_core_inputs(x, Wq, Wk, Wv, Wo, gamma, beta):
    """Host-side prep: per-(batch, head-group) input dicts."""
    import ml_dtypes
    bf16 = ml_dtypes.bfloat16
    B = x.shape[0]

    ii = np.arange(P)[:, None]   # j within block
    jj = np.arange(P)[None, :]   # i within block
    diag = (ii <= jj).astype(np.float32)     # causal:   j <= i
    mid = np.ones((P, P), np.float32)
    far = (jj < ii).astype(np.float32)       # window:   i - j < 256
    m01 = np.concatenate([diag, mid, far], axis=1)      # [128, 384]
    m01 = np.tile(m01, (1, 2)).astype(bf16)             # [128, 768]

    xts = [np.ascontiguousarray(x[b].T).astype(bf16) for b in range(B)]

    in_maps = []
    for b in range(B):
        for g in range(4):
            sl = slice(g * DG, (g + 1) * DG)
            sq = np.float32(1.0 / np.sqrt(DK))
            wq_g = (gamma[:, None] * Wq[:, sl] * sq)
            wk_g = gamma[:, None] * Wk[:, sl]
            wv_g = gamma[:, None] * Wv[:, sl]
            bq_g = ((beta @ Wq[:, sl]) * sq).astype(np.float32)
            bk_g = (beta @ Wk[:, sl]).astype(np.float32)
            bv_g = (beta @ Wv[:, sl]).astype(np.float32)
            in_maps.append({
                "xt": xts[b],
                "wq": np.ascontiguousarray(wq_g).astype(bf16),
                "wk": np.ascontiguousarray(wk_g).astype(bf16),
                "wv": np.ascontiguousarray(wv_g).astype(bf16),
                "wo": np.ascontiguousarray(Wo[sl, :]).astype(bf16),
                "bq": np.ascontiguousarray(bq_g.reshape(2, P).T),
                "bk": np.ascontiguousarray(bk_g.reshape(2, P).T),
                "bvb": np.tile(bv_g[None, :], (P, 1)),
                "m01": m01,
            })
    return in_maps


def _ntff_hook(so_path="/opt/axon/libaxon_pjrt.so"):
    import contextlib
    import ctypes

    lib = ctypes.CDLL(so_path)
    lib.axon_start_nrt_profile.argtypes = [
        ctypes.POINTER(ctypes.c_int64), ctypes.c_size_t]
    lib.axon_start_nrt_profile.restype = ctypes.c_int64
    lib.axon_stop_nrt_profile.argtypes = [ctypes.c_char_p]
    lib.axon_stop_nrt_profile.restype = ctypes.c_int64

    @contextlib.contextmanager
    def _hook(output_dir, device_ids):
        import jax
        jax.devices()
        if device_ids:
            ids = (ctypes.c_int64 * len(device_ids))(*device_ids)
            rc = lib.axon_start_nrt_profile(ids, len(device_ids))
        else:
            rc = lib.axon_start_nrt_profile(None, 0)
        if rc != 0:
            raise RuntimeError(f"axon_start_nrt_profile rc={rc}")
        try:
            yield
        finally:
            n = lib.axon_stop_nrt_profile(str(output_dir).encode())
            print(f"profile: {n} file(s) written to {output_dir}")

    return _hook


def _run_traced(nc, in_maps, trace_dir=None):
    """Execute via PJRT with NTFF capture; return BassKernelResults with
    exec_time_ns and a perfetto trace."""
    import glob
    import tempfile

    import gauge.profiler
    from concourse import bass2jax, bass_utils
    from concourse._compat import FishPath

    neff_dir = trace_dir or tempfile.mkdtemp(prefix="trn_trace_")
    hook = _ntff_hook()
    with hook(neff_dir, [0]):
        results = bass2jax.run_bass_via_pjrt(nc, in_maps, n_cores=len(in_maps))

    ntffs = glob.glob(os.path.join(neff_dir, "*_body*.ntff"))
    if not ntffs:
        print(f"no ntffs in {neff_dir}: {os.listdir(neff_dir)}")
        return bass_utils.BassKernelResults(
            results=results, instructions_and_trace=None,
            profile_json=None, exec_time_ns=None)

    profile = gauge.profiler.Profile(
        profile_path=FishPath(neff_dir),
        kernel_dev_mode=True,
        profile_on_exit=False,
        bass_kernel=nc.m,
        offline_processing=True,
        fname="*_body*",
        metadata={},
    )
    return bass_utils._process_ntff_profile(
        profile, neff_dir, nc, list(range(len(in_maps))),
        None, False, {}, trace_events=False,
    ).as_bass_kernel_results(results)


def kernel(x, Wq, Wk, Wv, Wo, bo, gamma, beta, trace=False):
    global LAST_PROFILE
    x = np.asarray(x, dtype=np.float32)
    Wq, Wk, Wv, Wo = (np.asarray(a, dtype=np.float32) for a in (Wq, Wk, Wv, Wo))
    bo = np.asarray(bo, dtype=np.float32)
    gamma = np.asarray(gamma, dtype=np.float32)
    beta = np.asarray(beta, dtype=np.float32)

    nc = build_nc()
    in_maps = _prep_core_inputs(x, Wq, Wk, Wv, Wo, gamma, beta)
    if trace:
        res = _run_traced(nc, in_maps)
    else:
        res = run_bass_kernel_spmd(nc, in_maps, core_ids=list(range(8)))
    LAST_PROFILE = {"exec_time_ns": res.exec_time_ns}

    B = x.shape[0]
    out = np.empty_like(x)
    for b in range(B):
        acc = x[b] + bo[None, :]
        for g in range(4):
            acc = acc + res.results[b * 4 + g]["partial"].astype(np.float32)
        out[b] = acc
    return out


# revision 14
# speedup vs baseline: 2.1334x; 2.1334x over previous
"""Local causal (sliding-window) attention block on 8 TRN2 NeuronCores.

Reference computation (per batch b):
    h = LayerNorm(x) * gamma + beta
    Q = h@Wq, K = h@Wk, V = h@Wv          (heads: 16 x 64)
    S = QK^T/sqrt(dk) masked to causal band of width 256
    out = x + softmax(S)@V @ Wo + bo

Sharding: 8 cores = 2 batches x 4 head-groups (4 heads each).
Each core receives x^T (feature-major, bf16) for its batch, computes LN
stats via ones-matmuls (mu, E[x^2] broadcast over partitions), centers
and scales x^T on DVE, projects Q^T/K^T (feature-major) and V
(token-major), then does banded attention in the transposed-score
domain: S^T = K_blk^T Q (so softmaxed probabilities are directly in
the layout the P^T V matmul needs -- no PE transposes).  The softmax
denominator comes from ones-vector matmuls over P^T, is reciprocal'd
and broadcast back over partitions with a K=1 matmul, and the
normalization is applied per-head on the attention output.  The
partial out-projection  attn_g @ Wo[g]  is written token-major in
bf16.  Host reduces: out[b] = x[b] + sum_g partial[b,g] + bo.

Everything on the PE runs in bf16 (FWL fast weight loads); gamma (and
1/sqrt(dk) for Q) are folded into the projection weights on the host;
beta enters via folded bias vectors beta@W.
"""

import os

import numpy as np

import concourse.bass as bass
import concourse.tile as tile
from concourse import bacc, mybir
from concourse.bass_utils import run_bass_kernel_spmd

F32 = mybir.dt.float32
BF16 = mybir.dt.bfloat16

T = 2048          # tokens per batch
D = 1024          # model dim
HG = 4            # heads per core
DK = 64           # head dim
DG = HG * DK      # head-group feature width (256)
WIN = 256         # attention window
P = 128           # partitions
NT = T // P       # 16 token tiles
KC = D // P       # 8 feature chunks
NQ = 512          # projection tile width
NTQ = T // NQ     # 4
LN_EPS = 1e-5

# filled by test.py via run(trace=True)
LAST_PROFILE = {}


def _body(tc):
    nc = tc.nc
    with nc.allow_low_precision(reason="bf16 pipeline; rel-err budget 2e-2"):
        _body_inner(tc)


def _body_inner(tc):
    nc = tc.nc

    xt = nc.dram_tensor("xt", [D, T], BF16, kind="ExternalInput").ap()
    wq = nc.dram_tensor("wq", [D, DG], BF16, kind="ExternalInput").ap()
    wk = nc.dram_tensor("wk", [D, DG], BF16, kind="ExternalInput").ap()
    wv = nc.dram_tensor("wv", [D, DG], BF16, kind="ExternalInput").ap()
    wo = nc.dram_tensor("wo", [DG, D], BF16, kind="ExternalInput").ap()
    bq = nc.dram_tensor("bq", [P, 2], F32, kind="ExternalInput").ap()
    bk = nc.dram_tensor("bk", [P, 2], F32, kind="ExternalInput").ap()
    bvb = nc.dram_tensor("bvb", [P, DG], F32, kind="ExternalInput").ap()
    m01 = nc.dram_tensor("m01", [P, 2 * 3 * P], BF16, kind="ExternalInput").ap()
    partial = nc.dram_tensor("partial", [T, D], BF16, kind="ExternalOutput").ap()

    with (
        tc.tile_pool(name="consts", bufs=1) as consts,
        tc.tile_pool(name="big", bufs=1) as big,
    ):
        # ---- resident SBUF tensors ----
        xc = [consts.tile([P, T], BF16, tag=f"x{c}", name=f"x{c}")
              for c in range(KC)]
        for c in range(KC):
            nc.sync.dma_start(out=xc[c], in_=xt[c * P:(c + 1) * P, :])

        wq_sb = consts.tile([P, KC, DG], BF16, tag="wq")
        wk_sb = consts.tile([P, KC, DG], BF16, tag="wk")
        wv_sb = consts.tile([P, KC, DG], BF16, tag="wv")
        wo_sb = consts.tile([P, 2, D], BF16, tag="wo")
        bq_sb = consts.tile([P, 2], F32, tag="bq")
        bk_sb = consts.tile([P, 2], F32, tag="bk")
        bvb_sb = consts.tile([P, DG], F32, tag="bvb")
        m01_sb = consts.tile([P, 2, 3 * P], BF16, tag="m01")
        nc.sync.dma_start(out=wq_sb, in_=wq.rearrange("(c p) n -> p c n", p=P))
        nc.sync.dma_start(out=wk_sb, in_=wk.rearrange("(c p) n -> p c n", p=P))
        nc.sync.dma_start(out=wv_sb, in_=wv.rearrange("(c p) n -> p c n", p=P))
        nc.sync.dma_start(out=wo_sb, in_=wo.rearrange("(c p) n -> p c n", p=P))
        nc.sync.dma_start(out=bq_sb, in_=bq)
        nc.sync.dma_start(out=bk_sb, in_=bk)
        nc.sync.dma_start(out=bvb_sb, in_=bvb)
        nc.sync.dma_start(out=m01_sb, in_=m01.rearrange("p (h n) -> p h n", h=2))

        ones_mu = consts.tile([P, P], BF16, tag="ones_mu")
        onescol = consts.tile([P, 1], BF16, tag="onescol")
        ones1 = consts.tile([1, P], BF16, tag="ones1")
        eps_sb = consts.tile([P, 1], F32, tag="eps")
        nc.vector.memset(ones_mu, 1.0 / D)
        nc.vector.memset(onescol, 1.0)
        nc.vector.memset(ones1, 1.0)
        nc.vector.memset(eps_sb, LN_EPS)

        mu_sb = big.tile([P, T], BF16, tag="mu")
        rstd_sb = big.tile([P, T], BF16, tag="rstd")
        xs = [big.tile([P, T], BF16, tag=f"xs{c}", name=f"xs{c}")
              for c in range(KC)]
        qt_sb = big.tile([P, 2, T], BF16, tag="qt")
        kt_sb = big.tile([P, 2, T], BF16, tag="kt")
        v_sb = big.tile([P, NT, DG], BF16, tag="v")
        ot_sb = big.tile([P, 2, T], BF16, tag="ot")

        # ==== Phases A+B: LN stats (via matmul), xs, Q/K/V projections ====
        # Processed in two token halves so stats matmuls of half h+1
        # overlap the DVE evac/xs work of half h (keeps PE dense/warm).
        from concourse.dve_ops import (
            RECIP_APPROX_FAST_CONSTS,
            RECIPROCAL_APPROX_FAST,
        )
        rc = RECIP_APPROX_FAST_CONSTS
        TH = T // 2
        with (
            tc.tile_pool(name="statp", bufs=1, space="PSUM") as statp,
            tc.tile_pool(name="sqp", bufs=3) as sqp,
            tc.tile_pool(name="stmp", bufs=2) as stmp,
            tc.tile_pool(name="qkp", bufs=2, space="PSUM") as qkp,
            tc.tile_pool(name="vp", bufs=2, space="PSUM") as vp,
        ):
            for h in range(2):
                hsl = slice(h * TH, (h + 1) * TH)
                mu_t = statp.tile([P, 2, NQ], F32, tag="mu", name=f"mu{h}")
                msq_t = statp.tile([P, 2, NQ], F32, tag="msq",
                                   name=f"msq{h}")
                for c in range(KC):
                    sqt = sqp.tile([P, TH], BF16, tag="sq")
                    nc.vector.tensor_mul(sqt, xc[c][:, hsl], xc[c][:, hsl])
                    for k in range(2):
                        nt = 2 * h + k
                        sl = slice(nt * NQ, (nt + 1) * NQ)
                        nc.tensor.matmul(
                            mu_t[:, k, :], ones_mu, xc[c][:, sl],
                            start=(c == 0), stop=(c == KC - 1),
                            skip_group_check=True,
                        )
                        nc.tensor.matmul(
                            msq_t[:, k, :], ones_mu,
                            sqt[:, k * NQ:(k + 1) * NQ],
                            start=(c == 0), stop=(c == KC - 1),
                            skip_group_check=True,
                        )
                for k in range(2):
                    nt = 2 * h + k
                    sl = slice(nt * NQ, (nt + 1) * NQ)
                    nc.vector.tensor_copy(mu_sb[:, sl], mu_t[:, k, :])
                    m2 = stmp.tile([P, NQ], F32, tag="m2")
                    nc.vector.tensor_mul(m2, mu_sb[:, sl], mu_sb[:, sl])
                    var = stmp.tile([P, NQ], F32, tag="var")
                    nc.vector.tensor_sub(var, msq_t[:, k, :], m2)
                    sd = stmp.tile([P, NQ], F32, tag="sd")
                    nc.scalar.activation(
                        out=sd, in_=var,
                        func=mybir.ActivationFunctionType.Sqrt,
                        bias=eps_sb, scale=1.0,
                    )
                    nc.vector._custom_dve(
                        RECIPROCAL_APPROX_FAST,
                        out=rstd_sb[:, sl], in0=sd,
                        s0=rc["s0"], s1=rc["s1"], imm2=rc["imm2"],
                    )
                # xs = (x - mu) * rstd   (normalized input, bf16)
                for c in range(KC):
                    nc.vector.tensor_sub(
                        xs[c][:, hsl], xc[c][:, hsl], mu_sb[:, hsl])
                    nc.vector.tensor_mul(
                        xs[c][:, hsl], xs[c][:, hsl], rstd_sb[:, hsl])
                # Q/K for this half
                for k in range(2):
                    nt = 2 * h + k
                    sl = slice(nt * NQ, (nt + 1) * NQ)
                    for oc in range(2):
                        for w_sb, dst, b_sb in ((wq_sb, qt_sb, bq_sb),
                                                (wk_sb, kt_sb, bk_sb)):
                            ps = qkp.tile([P, NQ], F32, tag="ps")
                            for c in range(KC):
                                nc.tensor.matmul(
                                    ps,
                                    w_sb[:, c, oc * P:(oc + 1) * P],
                                    xs[c][:, sl],
                                    start=(c == 0), stop=(c == KC - 1),
                                )
                            nc.scalar.activation(
                                out=dst[:, oc, sl], in_=ps,
                                func=mybir.ActivationFunctionType.Identity,
                                bias=b_sb[:, oc:oc + 1], scale=1.0,
                            )
            for tb in range(NT):
                tsl = slice(tb * P, (tb + 1) * P)
                ps = vp.tile([P, DG], F32, tag="psv")
                for c in range(KC):
                    nc.tensor.matmul(
                        ps, xs[c][:, tsl], wv_sb[:, c, :],
                        start=(c == 0), stop=(c == KC - 1),
                    )
                nc.vector.tensor_add(v_sb[:, tb, :], ps, bvb_sb)

        # ====== Phase C+D: banded attention (transposed scores) + out ====
        with (
            tc.tile_pool(name="sp", bufs=3, space="PSUM") as sp,
            tc.tile_pool(name="avp", bufs=3, space="PSUM") as avp,
            tc.tile_pool(name="dps", bufs=2, space="PSUM") as dps,
            tc.tile_pool(name="pbp", bufs=8) as pbp,
            tc.tile_pool(name="dvp", bufs=3) as dvp,
            tc.tile_pool(name="obp", bufs=3) as obp,
        ):
            pbs = {}
            for qb in range(NT):
                qsl = slice(qb * P, (qb + 1) * P)
                for oc in range(2):
                    # scores S^T for key-block jb=qb against i in
                    # [qb*P, qb*P+ni), exp'd and 0/1-masked
                    jb = qb
                    i0 = jb * P
                    ni = min(3 * P, T - i0)
                    pb = pbp.tile([P, 2, 3 * P], BF16, tag="pb")
                    pbs[(oc, jb)] = pb
                    for hh in range(2):
                        p0 = hh * DK
                        st = sp.tile([P, NQ], F32, tag="st")
                        nc.tensor.matmul(
                            st[:, :ni],
                            kt_sb[p0:p0 + DK, oc, jb * P:(jb + 1) * P],
                            qt_sb[p0:p0 + DK, oc, i0:i0 + ni],
                            start=True, stop=True,
                        )
                        nc.scalar.activation(
                            out=pb[:, hh, :ni], in_=st[:, :ni],
                            func=mybir.ActivationFunctionType.Exp,
                        )
                    nc.vector.tensor_mul(
                        pb[:, :, :ni], pb[:, :, :ni], m01_sb[:, :, :ni])

                    # softmax denominator + P^T V for query-block qb
                    # (av cols 0-127 and den cols 128-383 share one bank)
                    jlo = max(0, qb - 2)
                    njb = qb - jlo + 1
                    av = avp.tile([P, 3, P], F32, tag="av")
                    for k, j in enumerate(range(jlo, qb + 1)):
                        pbj = pbs[(oc, j)]
                        c0 = (qb - j) * P
                        nc.tensor.matmul(
                            av[0:1, 1:3, :], onescol, pbj[:, :, c0:c0 + P],
                            start=(k == 0), stop=(k == njb - 1),
                            skip_group_check=True,
                        )
                    dinv = dvp.tile([1, 2, P], BF16, tag="dinv")
                    nc.vector._custom_dve(
                        RECIPROCAL_APPROX_FAST,
                        out=dinv, in0=av[0:1, 1:3, :],
                        s0=rc["s0"], s1=rc["s1"], imm2=rc["imm2"],
                    )
                    dinvb = dvp.tile([P, 2, P], BF16, tag="dinvb")
                    nc.gpsimd.partition_broadcast(dinvb, dinv)
                    for hh in range(2):
                        p0 = hh * DK
                        for k, j in enumerate(range(jlo, qb + 1)):
                            pbj = pbs[(oc, j)]
                            c0 = (qb - j) * P
                            nc.tensor.matmul(
                                av[p0:p0 + DK, 0, :],
                                v_sb[:, j, oc * P + p0:oc * P + p0 + DK],
                                pbj[:, hh, c0:c0 + P],
                                start=(k == 0), stop=(k == njb - 1),
                                skip_group_check=True,
                                tile_position=(0, p0),
                            )
                    for hh in range(2):
                        p0 = hh * DK
                        nc.vector.tensor_mul(
                            ot_sb[p0:p0 + DK, oc, qsl],
                            av[p0:p0 + DK, 0, :],
                            dinvb[p0:p0 + DK, hh, :],
                        )

                # out-projection for this token block
                for on in range(2):
                    ps = dps.tile([P, NQ], F32, tag="ps")
                    for kd in range(2):
                        nc.tensor.matmul(
                            ps,
                            ot_sb[:, kd, qsl],
                            wo_sb[:, kd, on * NQ:(on + 1) * NQ],
                            start=(kd == 0), stop=(kd == 1),
                        )
                    ob = obp.tile([P, NQ], BF16, tag="ob")
                    if on == 0:
                        nc.vector.tensor_copy(ob, ps)
                    else:
                        nc.scalar.activation(
                            out=ob, in_=ps,
                            func=mybir.ActivationFunctionType.Copy,
                        )
                    nc.sync.dma_start(
                        out=partial[qsl, on * NQ:(on + 1) * NQ], in_=ob)


def build_nc():
    nc = bacc.Bacc("TRN2", target_bir_lowering=False, debug=False,
                   num_devices=8)
    with tile.TileContext(nc) as tc:
        _body(tc)
    nc.compile()
    return nc


def _prep_core_inputs(x, Wq, Wk, Wv, Wo, gamma, beta):
    """Host-side prep: per-(batch, head-group) input dicts."""
    import ml_dtypes
    bf16 = ml_dtypes.bfloat16
    B = x.shape[0]

    ii = np.arange(P)[:, None]   # j within block
    jj = np.arange(P)[None, :]   # i within block
    diag = (ii <= jj).astype(np.float32)     # causal:   j <= i
    mid = np.ones((P, P), np.float32)
    far = (jj < ii).astype(np.float32)       # window:   i - j < 256
    m01 = np.concatenate([diag, mid, far], axis=1)      # [128, 384]
    m01 = np.tile(m01, (1, 2)).astype(bf16)             # [128, 768]

    xts = [np.ascontiguousarray(x[b].T).astype(bf16) for b in range(B)]

    in_maps = []
    for b in range(B):
        for g in range(4):
            sl = slice(g * DG, (g + 1) * DG)
            sq = np.float32(1.0 / np.sqrt(DK))
            wq_g = (gamma[:, None] * Wq[:, sl] * sq)
            wk_g = gamma[:, None] * Wk[:, sl]
            wv_g = gamma[:, None] * Wv[:, sl]
            bq_g = ((beta @ Wq[:, sl]) * sq).astype(np.float32)
            bk_g = (beta @ Wk[:, sl]).astype(np.float32)
            bv_g = (beta @ Wv[:, sl]).astype(np.float32)
            in_maps.append({
                "xt": xts[b],
                "wq": np.ascontiguousarray(wq_g).astype(bf16),
                "wk": np.ascontiguousarray(wk_g).astype(bf16),
                "wv": np.ascontiguousarray(wv_g).astype(bf16),
                "wo": np.ascontiguousarray(Wo[sl, :]).astype(bf16),
                "bq": np.ascontiguousarray(bq_g.reshape(2, P).T),
                "bk": np.ascontiguousarray(bk_g.reshape(2, P).T),
                "bvb": np.tile(bv_g[None, :], (P, 1)),
                "m01": m01,
            })
    return in_maps


def _ntff_hook(so_path="/opt/axon/libaxon_pjrt.so"):
    import contextlib
    import ctypes

    lib = ctypes.CDLL(so_path)
    lib.axon_start_nrt_profile.argtypes = [
        ctypes.POINTER(ctypes.c_int64), ctypes.c_size_t]
    lib.axon_start_nrt_profile.restype = ctypes.c_int64
    lib.axon_stop_nrt_profile.argtypes = [ctypes.c_char_p]
    lib.axon_stop_nrt_profile.restype = ctypes.c_int64

    @contextlib.contextmanager
    def _hook(output_dir, device_ids):
        import jax
        jax.devices()
        if device_ids:
            ids = (ctypes.c_int64 * len(device_ids))(*device_ids)
            rc = lib.axon_start_nrt_profile(ids, len(device_ids))
        else:
            rc = lib.axon_start_nrt_profile(None, 0)
        if rc != 0:
            raise RuntimeError(f"axon_start_nrt_profile rc={rc}")
        try:
            yield
        finally:
            n = lib.axon_stop_nrt_profile(str(output_dir).encode())
            print(f"profile: {n} file(s) written to {output_dir}")

    return _hook


def _run_traced(nc, in_maps, trace_dir=None):
    """Execute via PJRT with NTFF capture; return BassKernelResults with
    exec_time_ns and a perfetto trace."""
    import glob
    import tempfile

    import gauge.profiler
    from concourse import bass2jax, bass_utils
    from concourse._compat import FishPath

    neff_dir = trace_dir or tempfile.mkdtemp(prefix="trn_trace_")
    hook = _ntff_hook()
    with hook(neff_dir, [0]):
        results = bass2jax.run_bass_via_pjrt(nc, in_maps, n_cores=len(in_maps))

    ntffs = glob.glob(os.path.join(neff_dir, "*_body*.ntff"))
    if not ntffs:
        print(f"no ntffs in {neff_dir}: {os.listdir(neff_dir)}")
        return bass_utils.BassKernelResults(
            results=results, instructions_and_trace=None,
            profile_json=None, exec_time_ns=None)

    profile = gauge.profiler.Profile(
        profile_path=FishPath(neff_dir),
        kernel_dev_mode=True,
        profile_on_exit=False,
        bass_kernel=nc.m,
        offline_processing=True,
        fname="*_body*",
        metadata={},
    )
    return bass_utils._process_ntff_profile(
        profile, neff_dir, nc, list(range(len(in_maps))),
        None, False, {}, trace_events=False,
    ).as_bass_kernel_results(results)


def kernel(x, Wq, Wk, Wv, Wo, bo, gamma, beta, trace=False):
    global LAST_PROFILE
    x = np.asarray(x, dtype=np.float32)
    Wq, Wk, Wv, Wo = (np.asarray(a, dtype=np.float32) for a in (Wq, Wk, Wv, Wo))
    bo = np.asarray(bo, dtype=np.float32)
    gamma = np.asarray(gamma, dtype=np.float32)
    beta = np.asarray(beta, dtype=np.float32)

    nc = build_nc()
    in_maps = _prep_core_inputs(x, Wq, Wk, Wv, Wo, gamma, beta)
    if trace:
        res = _run_traced(nc, in_maps)
    else:
        res = run_bass_kernel_spmd(nc, in_maps, core_ids=list(range(8)))
    LAST_PROFILE = {"exec_time_ns": res.exec_time_ns}

    B = x.shape[0]
    out = np.empty_like(x)
    for b in range(B):
        acc = x[b] + bo[None, :]
        for g in range(4):
            acc = acc + res.results[b * 4 + g]["partial"].astype(np.float32)
        out[b] = acc
    return out


# revision 18
# speedup vs baseline: 2.4914x; 1.1678x over previous
"""Local causal (sliding-window) attention block on 8 TRN2 NeuronCores.

Reference computation (per batch b):
    h = LayerNorm(x) * gamma + beta
    Q = h@Wq, K = h@Wk, V = h@Wv          (heads: 16 x 64)
    S = QK^T/sqrt(dk) masked to causal band of width 256
    out = x + softmax(S)@V @ Wo + bo

Sharding: 8 cores = 2 batches x 4 head-groups (4 heads each).
Each core receives x^T (feature-major, bf16) for its batch, computes LN
stats via ones-matmuls (mu, E[x^2] broadcast over partitions), centers
and scales x^T on DVE, projects Q^T/K^T (feature-major) and V
(token-major), then does banded attention in the transposed-score
domain: S^T = K_blk^T Q (so softmaxed probabilities are directly in
the layout the P^T V matmul needs -- no PE transposes).  The softmax
denominator comes from ones-vector matmuls over P^T, is reciprocal'd
and broadcast back over partitions with a K=1 matmul, and the
normalization is applied per-head on the attention output.  The
partial out-projection  attn_g @ Wo[g]  is written token-major in
bf16.  Host reduces: out[b] = x[b] + sum_g partial[b,g] + bo.

Everything on the PE runs in bf16 (FWL fast weight loads); gamma (and
1/sqrt(dk) for Q) are folded into the projection weights on the host;
beta enters via folded bias vectors beta@W.
"""

import os

import numpy as np

import concourse.bass as bass
import concourse.tile as tile
from concourse import bacc, mybir
from concourse.bass_utils import run_bass_kernel_spmd

F32 = mybir.dt.float32
BF16 = mybir.dt.bfloat16

T = 2048          # tokens per batch
D = 1024          # model dim
HG = 4            # heads per core
DK = 64           # head dim
DG = HG * DK      # head-group feature width (256)
WIN = 256         # attention window
P = 128           # partitions
NT = T // P       # 16 token tiles
KC = D // P       # 8 feature chunks
NQ = 512          # projection tile width
NTQ = T // NQ     # 4
LN_EPS = 1e-5

# filled by test.py via run(trace=True)
LAST_PROFILE = {}


def _body(tc):
    nc = tc.nc
    with nc.allow_low_precision(reason="bf16 pipeline; rel-err budget 2e-2"):
        _body_inner(tc)


def _body_inner(tc):
    nc = tc.nc

    xt = nc.dram_tensor("xt", [D, T], BF16, kind="ExternalInput").ap()
    wq = nc.dram_tensor("wq", [D, DG], BF16, kind="ExternalInput").ap()
    wk = nc.dram_tensor("wk", [D, DG], BF16, kind="ExternalInput").ap()
    wv = nc.dram_tensor("wv", [D, DG], BF16, kind="ExternalInput").ap()
    wo = nc.dram_tensor("wo", [DG, D], BF16, kind="ExternalInput").ap()
    bq = nc.dram_tensor("bq", [P, 2], F32, kind="ExternalInput").ap()
    bk = nc.dram_tensor("bk", [P, 2], F32, kind="ExternalInput").ap()
    bvb = nc.dram_tensor("bvb", [P, DG], F32, kind="ExternalInput").ap()
    m01 = nc.dram_tensor("m01", [P, 2 * 3 * P], BF16, kind="ExternalInput").ap()
    partial = nc.dram_tensor("partial", [T, D], BF16, kind="ExternalOutput").ap()

    with (
        tc.tile_pool(name="consts", bufs=1) as consts,
        tc.tile_pool(name="big", bufs=1) as big,
    ):
        # ---- resident SBUF tensors ----
        xc = [consts.tile([P, T], BF16, tag=f"x{c}", name=f"x{c}")
              for c in range(KC)]
        HT = T // 2
        for c in range(KC):
            nc.sync.dma_start(out=xc[c][:, :HT], in_=xt[c * P:(c + 1) * P, :HT])

        wq_sb = consts.tile([P, KC, DG], BF16, tag="wq")
        wk_sb = consts.tile([P, KC, DG], BF16, tag="wk")
        wv_sb = consts.tile([P, KC, DG], BF16, tag="wv")
        wo_sb = consts.tile([P, 2, D], BF16, tag="wo")
        bq_sb = consts.tile([P, 2], F32, tag="bq")
        bk_sb = consts.tile([P, 2], F32, tag="bk")
        bvb_sb = consts.tile([P, DG], F32, tag="bvb")
        m01_sb = consts.tile([P, 2, 3 * P], BF16, tag="m01")
        nc.sync.dma_start(out=wq_sb, in_=wq.rearrange("(c p) n -> p c n", p=P))
        nc.sync.dma_start(out=wk_sb, in_=wk.rearrange("(c p) n -> p c n", p=P))
        for c in range(KC):
            nc.sync.dma_start(out=xc[c][:, HT:], in_=xt[c * P:(c + 1) * P, HT:])
        nc.sync.dma_start(out=wv_sb, in_=wv.rearrange("(c p) n -> p c n", p=P))
        nc.sync.dma_start(out=wo_sb, in_=wo.rearrange("(c p) n -> p c n", p=P))
        nc.sync.dma_start(out=bq_sb, in_=bq)
        nc.sync.dma_start(out=bk_sb, in_=bk)
        nc.sync.dma_start(out=bvb_sb, in_=bvb)
        nc.sync.dma_start(out=m01_sb, in_=m01.rearrange("p (h n) -> p h n", h=2))

        ones_mu = consts.tile([P, P], BF16, tag="ones_mu")
        onescol = consts.tile([P, 1], BF16, tag="onescol")
        eps_sb = consts.tile([P, 1], F32, tag="eps")
        nc.vector.memset(ones_mu, 1.0 / D)
        nc.vector.memset(onescol, 1.0)
        nc.vector.memset(eps_sb, LN_EPS)

        mu_sb = big.tile([P, T], BF16, tag="mu")
        rstd_sb = big.tile([P, T], BF16, tag="rstd")
        xs = [big.tile([P, T], BF16, tag=f"xs{c}", name=f"xs{c}")
              for c in range(KC)]
        qt_sb = big.tile([P, 2, T], BF16, tag="qt")
        kt_sb = big.tile([P, 2, T], BF16, tag="kt")
        v_sb = big.tile([P, NT, DG], BF16, tag="v")
        ot_sb = big.tile([P, 2, T], BF16, tag="ot")

        # ==== Phases A+B: LN stats (via matmul), xs, Q/K/V projections ====
        # Processed in two token halves so stats matmuls of half h+1
        # overlap the DVE evac/xs work of half h (keeps PE dense/warm).
        from concourse.dve_ops import (
            RECIP_APPROX_FAST_CONSTS,
            RECIPROCAL_APPROX_FAST,
        )
        rc = RECIP_APPROX_FAST_CONSTS
        TH = T // 2
        with (
            tc.tile_pool(name="statp", bufs=1, space="PSUM") as statp,
            tc.tile_pool(name="sqp", bufs=3) as sqp,
            tc.tile_pool(name="stmp", bufs=2) as stmp,
            tc.tile_pool(name="qkp", bufs=2, space="PSUM") as qkp,
        ):
            for h in range(2):
                hsl = slice(h * TH, (h + 1) * TH)
                mu_t = statp.tile([P, 2, NQ], F32, tag="mu", name=f"mu{h}")
                msq_t = statp.tile([P, 2, NQ], F32, tag="msq",
                                   name=f"msq{h}")
                for c in range(KC):
                    sqt = sqp.tile([P, TH], BF16, tag="sq")
                    nc.vector.tensor_mul(sqt, xc[c][:, hsl], xc[c][:, hsl])
                    for k in range(2):
                        nt = 2 * h + k
                        sl = slice(nt * NQ, (nt + 1) * NQ)
                        nc.tensor.matmul(
                            mu_t[:, k, :], ones_mu, xc[c][:, sl],
                            start=(c == 0), stop=(c == KC - 1),
                            skip_group_check=True,
                        )
                        nc.tensor.matmul(
                            msq_t[:, k, :], ones_mu,
                            sqt[:, k * NQ:(k + 1) * NQ],
                            start=(c == 0), stop=(c == KC - 1),
                            skip_group_check=True,
                        )
                for k in range(2):
                    nt = 2 * h + k
                    sl = slice(nt * NQ, (nt + 1) * NQ)
                    nc.vector.tensor_copy(mu_sb[:, sl], mu_t[:, k, :])
                    m2 = stmp.tile([P, NQ], F32, tag="m2")
                    nc.vector.tensor_mul(m2, mu_sb[:, sl], mu_sb[:, sl])
                    var = stmp.tile([P, NQ], F32, tag="var")
                    nc.vector.tensor_sub(var, msq_t[:, k, :], m2)
                    sd = stmp.tile([P, NQ], F32, tag="sd")
                    nc.scalar.activation(
                        out=sd, in_=var,
                        func=mybir.ActivationFunctionType.Sqrt,
                        bias=eps_sb, scale=1.0,
                    )
                    nc.vector._custom_dve(
                        RECIPROCAL_APPROX_FAST,
                        out=rstd_sb[:, sl], in0=sd,
                        s0=rc["s0"], s1=rc["s1"], imm2=rc["imm2"],
                    )
                # xs = (x - mu) * rstd   (normalized input, bf16)
                for c in range(KC):
                    nc.vector.tensor_sub(
                        xs[c][:, hsl], xc[c][:, hsl], mu_sb[:, hsl])
                    nc.vector.tensor_mul(
                        xs[c][:, hsl], xs[c][:, hsl], rstd_sb[:, hsl])
                # Q/K for this half
                for k in range(2):
                    nt = 2 * h + k
                    sl = slice(nt * NQ, (nt + 1) * NQ)
                    for oc in range(2):
                        for w_sb, dst, b_sb in ((wq_sb, qt_sb, bq_sb),
                                                (wk_sb, kt_sb, bk_sb)):
                            ps = qkp.tile([P, NQ], F32, tag="ps")
                            for c in range(KC):
                                nc.tensor.matmul(
                                    ps,
                                    w_sb[:, c, oc * P:(oc + 1) * P],
                                    xs[c][:, sl],
                                    start=(c == 0), stop=(c == KC - 1),
                                )
                            nc.scalar.activation(
                                out=dst[:, oc, sl], in_=ps,
                                func=mybir.ActivationFunctionType.Identity,
                                bias=b_sb[:, oc:oc + 1], scale=1.0,
                            )
        # ====== Phase C+D: banded attention (transposed scores) + out ====
        # V-projection for token block tb is interleaved at qb = tb - 2 so
        # its PSUM slots rotate with the out-projection's (shared pool).
        with (
            tc.tile_pool(name="sp", bufs=3, space="PSUM") as sp,
            tc.tile_pool(name="avp", bufs=2, space="PSUM") as avp,
            tc.tile_pool(name="dps", bufs=3, space="PSUM") as dps,
            tc.tile_pool(name="pbp", bufs=8) as pbp,
            tc.tile_pool(name="dvp", bufs=3) as dvp,
            tc.tile_pool(name="obp", bufs=3) as obp,
        ):
            def emit_v(tb):
                tsl = slice(tb * P, (tb + 1) * P)
                ps = dps.tile([P, NQ], F32, tag="ps", name=f"psv{tb}")
                for c in range(KC):
                    nc.tensor.matmul(
                        ps[:, :DG], xs[c][:, tsl], wv_sb[:, c, :],
                        start=(c == 0), stop=(c == KC - 1),
                    )
                nc.vector.tensor_add(v_sb[:, tb, :], ps[:, :DG], bvb_sb)

            emit_v(0)
            emit_v(1)
            pbs = {}
            for qb in range(NT):
                if qb + 2 < NT:
                    emit_v(qb + 2)
                qsl = slice(qb * P, (qb + 1) * P)
                for oc in range(2):
                    # scores S^T for key-block jb=qb against i in
                    # [qb*P, qb*P+ni), exp'd and 0/1-masked
                    jb = qb
                    i0 = jb * P
                    ni = min(3 * P, T - i0)
                    pb = pbp.tile([P, 2, 3 * P], BF16, tag="pb")
                    pbs[(oc, jb)] = pb
                    for hh in range(2):
                        p0 = hh * DK
                        st = sp.tile([P, NQ], F32, tag="st")
                        nc.tensor.matmul(
                            st[:, :ni],
                            kt_sb[p0:p0 + DK, oc, jb * P:(jb + 1) * P],
                            qt_sb[p0:p0 + DK, oc, i0:i0 + ni],
                            start=True, stop=True,
                        )
                        nc.scalar.activation(
                            out=pb[:, hh, :ni], in_=st[:, :ni],
                            func=mybir.ActivationFunctionType.Exp,
                        )
                    nc.vector.tensor_mul(
                        pb[:, :, :ni], pb[:, :, :ni], m01_sb[:, :, :ni])

                    # softmax denominator + P^T V for query-block qb
                    # (av cols 0-127 and den cols 128-383 share one bank)
                    jlo = max(0, qb - 2)
                    njb = qb - jlo + 1
                    av = avp.tile([P, 3, P], F32, tag="av")
                    for k, j in enumerate(range(jlo, qb + 1)):
                        pbj = pbs[(oc, j)]
                        c0 = (qb - j) * P
                        nc.tensor.matmul(
                            av[0:1, 1:3, :], onescol, pbj[:, :, c0:c0 + P],
                            start=(k == 0), stop=(k == njb - 1),
                            skip_group_check=True,
                        )
                    dinv = dvp.tile([1, 2, P], BF16, tag="dinv")
                    nc.vector._custom_dve(
                        RECIPROCAL_APPROX_FAST,
                        out=dinv, in0=av[0:1, 1:3, :],
                        s0=rc["s0"], s1=rc["s1"], imm2=rc["imm2"],
                    )
                    dinvb = dvp.tile([P, 2, P], BF16, tag="dinvb")
                    nc.gpsimd.partition_broadcast(dinvb, dinv)
                    for hh in range(2):
                        p0 = hh * DK
                        for k, j in enumerate(range(jlo, qb + 1)):
                            pbj = pbs[(oc, j)]
                            c0 = (qb - j) * P
                            nc.tensor.matmul(
                                av[p0:p0 + DK, 0, :],
                                v_sb[:, j, oc * P + p0:oc * P + p0 + DK],
                                pbj[:, hh, c0:c0 + P],
                                start=(k == 0), stop=(k == njb - 1),
                                skip_group_check=True,
                                tile_position=(0, p0),
                            )
                    for hh in range(2):
                        p0 = hh * DK
                        nc.vector.tensor_mul(
                            ot_sb[p0:p0 + DK, oc, qsl],
                            av[p0:p0 + DK, 0, :],
                            dinvb[p0:p0 + DK, hh, :],
                        )

                # out-projection for this token block
                for on in range(2):
                    ps = dps.tile([P, NQ], F32, tag="ps")
                    for kd in range(2):
                        nc.tensor.matmul(
                            ps,
                            ot_sb[:, kd, qsl],
                            wo_sb[:, kd, on * NQ:(on + 1) * NQ],
                            start=(kd == 0), stop=(kd == 1),
                        )
                    ob = obp.tile([P, NQ], BF16, tag="ob")
                    if on == 0:
                        nc.vector.tensor_copy(ob, ps)
                    else:
                        nc.scalar.activation(
                            out=ob, in_=ps,
                            func=mybir.ActivationFunctionType.Copy,
                        )
                    nc.sync.dma_start(
                        out=partial[qsl, on * NQ:(on + 1) * NQ], in_=ob)


def build_nc():
    nc = bacc.Bacc("TRN2", target_bir_lowering=False, debug=False,
                   num_devices=8)
    with tile.TileContext(nc) as tc:
        _body(tc)
    nc.compile()
    return nc


def _prep_core_inputs(x, Wq, Wk, Wv, Wo, gamma, beta):
    """Host-side prep: per-(batch, head-group) input dicts."""
    import ml_dtypes
    bf16 = ml_dtypes.bfloat16
    B = x.shape[0]

    ii = np.arange(P)[:, None]   # j within block
    jj = np.arange(P)[None, :]   # i within block
    diag = (ii <= jj).astype(np.float32)     # causal:   j <= i
    mid = np.ones((P, P), np.float32)
    far = (jj < ii).astype(np.float32)       # window:   i - j < 256
    m01 = np.concatenate([diag, mid, far], axis=1)      # [128, 384]
    m01 = np.tile(m01, (1, 2)).astype(bf16)             # [128, 768]

    xts = [np.ascontiguousarray(x[b].T).astype(bf16) for b in range(B)]

    in_maps = []
    for b in range(B):
        for g in range(4):
            sl = slice(g * DG, (g + 1) * DG)
            sq = np.float32(1.0 / np.sqrt(DK))
            wq_g = (gamma[:, None] * Wq[:, sl] * sq)
            wk_g = gamma[:, None] * Wk[:, sl]
            wv_g = gamma[:, None] * Wv[:, sl]
            bq_g = ((beta @ Wq[:, sl]) * sq).astype(np.float32)
            bk_g = (beta @ Wk[:, sl]).astype(np.float32)
            bv_g = (beta @ Wv[:, sl]).astype(np.float32)
            in_maps.append({
                "xt": xts[b],
                "wq": np.ascontiguousarray(wq_g).astype(bf16),
                "wk": np.ascontiguousarray(wk_g).astype(bf16),
                "wv": np.ascontiguousarray(wv_g).astype(bf16),
                "wo": np.ascontiguousarray(Wo[sl, :]).astype(bf16),
                "bq": np.ascontiguousarray(bq_g.reshape(2, P).T),
                "bk": np.ascontiguousarray(bk_g.reshape(2, P).T),
                "bvb": np.tile(bv_g[None, :], (P, 1)),
                "m01": m01,
            })
    return in_maps


def _ntff_hook(so_path="/opt/axon/libaxon_pjrt.so"):
    import contextlib
    import ctypes

    lib = ctypes.CDLL(so_path)
    lib.axon_start_nrt_profile.argtypes = [
        ctypes.POINTER(ctypes.c_int64), ctypes.c_size_t]
    lib.axon_start_nrt_profile.restype = ctypes.c_int64
    lib.axon_stop_nrt_profile.argtypes = [ctypes.c_char_p]
    lib.axon_stop_nrt_profile.restype = ctypes.c_int64

    @contextlib.contextmanager
    def _hook(output_dir, device_ids):
        import jax
        jax.devices()
        if device_ids:
            ids = (ctypes.c_int64 * len(device_ids))(*device_ids)
            rc = lib.axon_start_nrt_profile(ids, len(device_ids))
        else:
            rc = lib.axon_start_nrt_profile(None, 0)
        if rc != 0:
            raise RuntimeError(f"axon_start_nrt_profile rc={rc}")
        try:
            yield
        finally:
            n = lib.axon_stop_nrt_profile(str(output_dir).encode())
            print(f"profile: {n} file(s) written to {output_dir}")

    return _hook


def _run_traced(nc, in_maps, trace_dir=None):
    """Execute via PJRT with NTFF capture; return BassKernelResults with
    exec_time_ns and a perfetto trace."""
    import glob
    import tempfile

    import gauge.profiler
    from concourse import bass2jax, bass_utils
    from concourse._compat import FishPath

    neff_dir = trace_dir or tempfile.mkdtemp(prefix="trn_trace_")
    hook = _ntff_hook()
    with hook(neff_dir, [0]):
        results = bass2jax.run_bass_via_pjrt(nc, in_maps, n_cores=len(in_maps))

    ntffs = glob.glob(os.path.join(neff_dir, "*_body*.ntff"))
    if not ntffs:
        print(f"no ntffs in {neff_dir}: {os.listdir(neff_dir)}")
        return bass_utils.BassKernelResults(
            results=results, instructions_and_trace=None,
            profile_json=None, exec_time_ns=None)

    profile = gauge.profiler.Profile(
        profile_path=FishPath(neff_dir),
        kernel_dev_mode=True,
        profile_on_exit=False,
        bass_kernel=nc.m,
        offline_processing=True,
        fname="*_body*",
        metadata={},
    )
    return bass_utils._process_ntff_profile(
        profile, neff_dir, nc, list(range(len(in_maps))),
        None, False, {}, trace_events=False,
    ).as_bass_kernel_results(results)


def kernel(x, Wq, Wk, Wv, Wo, bo, gamma, beta, trace=False):
    global LAST_PROFILE
    x = np.asarray(x, dtype=np.float32)
    Wq, Wk, Wv, Wo = (np.asarray(a, dtype=np.float32) for a in (Wq, Wk, Wv, Wo))
    bo = np.asarray(bo, dtype=np.float32)
    gamma = np.asarray(gamma, dtype=np.float32)
    beta = np.asarray(beta, dtype=np.float32)

    nc = build_nc()
    in_maps = _prep_core_inputs(x, Wq, Wk, Wv, Wo, gamma, beta)
    if trace:
        res = _run_traced(nc, in_maps)
    else:
        res = run_bass_kernel_spmd(nc, in_maps, core_ids=list(range(8)))
    LAST_PROFILE = {"exec_time_ns": res.exec_time_ns}

    B = x.shape[0]
    out = np.empty_like(x)
    for b in range(B):
        acc = x[b] + bo[None, :]
        for g in range(4):
            acc = acc + res.results[b * 4 + g]["partial"].astype(np.float32)
        out[b] = acc
    return out
